# revision 49
# baseline (speedup 1.0000x reference)
"""Trainium2 Bass kernel: batched causal attention (B=4, S=4096, E=256, f32).

Sharding: 2 cores per batch element (4 pairs).  Within a pair, K/V rows are
split even/odd at 128-row tile granularity; both cores process all 4096 query
rows against their 2048 K/V rows.  The instruction stream is identical across
cores (pure SPMD, no collectives): each core ships its *unnormalized* partial
O^T (bf16) plus per-chunk exp-row-sum accumulators (bf16) to DRAM, and the
host merges the pair (add + normalize + transpose + bias) during unshard.

Device-side design notes:
  - Host pre-transposes and pre-casts X^T, Z^T, weights to bf16 -> no PE
    transposes and half the input DMA bytes.
  - The Q projection is folded into K ("K2" = Wq'^T K^T, with Wq' carrying
    the 1/sqrt(E) score scale): scores contract X^T directly against K2, so
    no Q-projection activations gate the attention pipeline.  The q-bias
    term bq.k varies only along k = the PSUM partition dim, so it rides the
    exp activation as a per-partition bias (bqk, via two 1-column matmuls
    per z-chunk).  The k-bias bk is dropped (softmax shift invariance).
  - Scores S^T[k,q] = K2(stationary) . X^T; chunk pairs (2m, 2m+1) run
    k-tiles in lockstep so one scalar-engine activation covers both score
    tiles.  The two single (c1-only) k-tiles run FIRST so the pair ends on
    dense full-width work.
  - Diagonal k-tiles are narrowed to the live 256..512 column range; the
    partially-masked leading 256-column block uses one parity-encoded mask.
  - Exp row-sums accumulate on DVE in bf16 into a memset-zeroed tile; the
    host does the final 128-partition reduction.
  - Projection "front" blocks (K/V matmuls + casts) are emitted well ahead
    of the "back" blocks (K2/bqk) that consume their DVE casts, which are
    themselves a pair ahead of the attention that consumes them.
"""

import numpy as np

B = 4
S = 4096
E = 256
SK = S // 2          # K/V rows per core
KT = SK // 128       # 16 local k-tiles
NCH = S // 512       # 8 q-chunks of 512
F = 512

_COMPILED = {}


def _build():
    import concourse.bass as bass
    import concourse.tile as tile
    from concourse import mybir, bacc

    from concourse.alu_op_type import AluOpType
    f32 = mybir.dt.float32
    bf16 = mybir.dt.bfloat16
    Exp = mybir.ActivationFunctionType.Exp
    Copy = mybir.ActivationFunctionType.Copy
    AluMult = AluOpType.mult
    AluAdd = AluOpType.add
    AluBypass = AluOpType.bypass

    nc = bacc.Bacc("TRN2", target_bir_lowering=False, debug=False,
                   enable_asserts=False, num_devices=1)

    xt_ext = nc.dram_tensor("xt", [2, 128, S], bf16, kind="ExternalInput")
    zt_ext = nc.dram_tensor("zt", [2, 128, SK], bf16, kind="ExternalInput")
    wq_ext = nc.dram_tensor("wq", [2, 128, E], bf16, kind="ExternalInput")
    wk_ext = nc.dram_tensor("wk", [2, 128, E], bf16, kind="ExternalInput")
    wv_ext = nc.dram_tensor("wv", [2, 128, E], bf16, kind="ExternalInput")
    # bqw = (bq/sqrt(E)) @ Wk : host-computed e-vector; bqk = bqw . Z^T
    bqw_ext = nc.dram_tensor("bqw", [128, 2], bf16, kind="ExternalInput")
    # diag mask: one parity-encoded 256-col pattern (keep iff u >= 128p + k)
    masks_ext = nc.dram_tensor("masks", [128, 256], bf16, kind="ExternalInput")
    out_ext = nc.dram_tensor("out", [NCH, 2, 128, F], bf16, kind="ExternalOutput")
    acc_ext = nc.dram_tensor("acc", [NCH, 128, F], bf16, kind="ExternalOutput")

    with tile.TileContext(nc) as tc:
        with tc.tile_pool(name="singles", bufs=1) as singles, \
             tc.tile_pool(name="pT", bufs=6) as pTp, \
             tc.tile_pool(name="accp", bufs=3) as accp, \
             tc.tile_pool(name="pop", bufs=4) as pop, \
             tc.tile_pool(name="dram", bufs=1, space="DRAM") as dram, \
             tc.tile_pool(name="ps_a", bufs=2, space="PSUM") as ps_a, \
             tc.tile_pool(name="ps_o", bufs=2, space="PSUM") as ps_o:

            # ---- persistent SBUF tensors ----------------------------------
            xT = singles.tile([128, 2, S], bf16, tag="xT")
            zT = singles.tile([128, 2, SK], bf16, tag="zT")
            kT = singles.tile([128, 2, SK], bf16, tag="kT")
            k2T = singles.tile([128, 2, SK], bf16, tag="k2T")
            v_sb = singles.tile([128, KT, E], bf16, tag="v_sb")
            wq_sb = singles.tile([128, 2, E], bf16, tag="wq_sb")
            wk_sb = singles.tile([128, 2, E], bf16, tag="wk_sb")
            wv_sb = singles.tile([128, 2, E], bf16, tag="wv_sb")
            bqw = singles.tile([128, 2], bf16, tag="bqw")
            bqk_row = singles.tile([1, SK], f32, tag="bqk_row")
            bqk_dram = dram.tile([SK], f32)
            bqk_t = singles.tile([128, KT], f32, tag="bqk_t")
            maskt = singles.tile([128, 256], bf16, tag="maskt")

            # ---- all input DMAs issued up front ---------------------------
            nc.scalar.dma_start(out=wk_sb[:],
                                in_=wk_ext.ap().rearrange("e p f -> p e f"))
            nc.scalar.dma_start(out=wv_sb[:],
                                in_=wv_ext.ap().rearrange("e p f -> p e f"))
            nc.gpsimd.dma_start(out=bqw[:], in_=bqw_ext[:])
            nc.gpsimd.dma_start(out=maskt[:], in_=masks_ext[:])
            nc.gpsimd.dma_start(out=wq_sb[:],
                                in_=wq_ext.ap().rearrange("e p f -> p e f"))
            # sync queue: first z half-slice split fine so the K projection
            # starts ASAP, then z/x interleaved in need order
            for et in range(2):
                nc.sync.dma_start(out=zT[:, et, 0:512],
                                  in_=zt_ext[et, :, 0:512])
            for et in range(2):
                nc.sync.dma_start(out=zT[:, et, 512:1024],
                                  in_=zt_ext[et, :, 512:1024])
            zx_order = [("x", 0), ("z", 1), ("x", 1), ("x", 2), ("x", 3)]
            for kind, h in zx_order:
                for et in range(2):
                    if kind == "z":
                        nc.sync.dma_start(
                            out=zT[:, et, 1024 * h:1024 * (h + 1)],
                            in_=zt_ext[et, :, 1024 * h:1024 * (h + 1)])
                    else:
                        nc.sync.dma_start(
                            out=xT[:, et, 1024 * h:1024 * (h + 1)],
                            in_=xt_ext[et, :, 1024 * h:1024 * (h + 1)])

            def front_block(sc):
                # K^T tile, exp-bias weights, and e^{bqk}-scaled natural-V
                # tile for z columns [512sc, 512(sc+1))
                psk = ps_a.tile([128, 2 * F], f32, tag="ps_a", name="psk")
                for g in range(2):
                    for et in range(2):
                        nc.tensor.matmul(psk[:, F * g:F * (g + 1)],
                                         wk_sb[:, et, 128 * g:128 * (g + 1)],
                                         zT[:, et, 512 * sc:512 * (sc + 1)],
                                         start=(et == 0), stop=(et == 1),
                                         skip_group_check=(g == 1))
                nc.vector.tensor_copy(out=kT[:, :, 512 * sc:512 * (sc + 1)],
                                      in_=psk[:])
                # bqk = bqw . Z^T, bounced through DRAM into [128, 4] layout
                psb = ps_a.tile([1, F], f32, tag="ps_a", name="psb",
                                padded_shape=[128, 2 * F])
                for et in range(2):
                    nc.tensor.matmul(psb[:], bqw[:, et:et + 1],
                                     zT[:, et, 512 * sc:512 * (sc + 1)],
                                     start=(et == 0), stop=(et == 1))
                nc.vector.tensor_copy(out=bqk_row[:, 512 * sc:512 * (sc + 1)],
                                      in_=psb[:])
                nc.gpsimd.dma_start(
                    out=bqk_dram[512 * sc:512 * (sc + 1)],
                    in_=bqk_row[0:1, 512 * sc:512 * (sc + 1)])
                nc.gpsimd.dma_start(
                    out=bqk_t[:, 4 * sc:4 * (sc + 1)],
                    in_=bqk_dram[512 * sc:512 * (sc + 1)].rearrange(
                        "(l p) -> p l", p=128))
                psv = ps_a.tile([128, 4, E], f32, tag="ps_a", name="psv",
                                padded_shape=[128, 4, E])
                for t in range(4):
                    base = 512 * sc + 128 * t
                    for et in range(2):
                        nc.tensor.matmul(psv[:, t, :],
                                         zT[:, et, base:base + 128],
                                         wv_sb[:, et, :],
                                         start=(et == 0), stop=(et == 1),
                                         skip_group_check=(t > 0))
                nc.vector.tensor_copy(out=v_sb[:, 4 * sc:4 * sc + 4, :],
                                      in_=psv[:])

            def back_block(sc):
                # K2 = Wq'^T K^T (scores then contract X directly)
                psk2 = ps_a.tile([128, 2 * F], f32, tag="ps_a", name="psk2")
                for g in range(2):
                    for ft in range(2):
                        nc.tensor.matmul(psk2[:, F * g:F * (g + 1)],
                                         wq_sb[:, ft, 128 * g:128 * (g + 1)],
                                         kT[:, ft, 512 * sc:512 * (sc + 1)],
                                         start=(ft == 0), stop=(ft == 1),
                                         skip_group_check=(g == 1))
                nc.vector.tensor_copy(out=k2T[:, :, 512 * sc:512 * (sc + 1)],
                                      in_=psk2[:])

            def attn_pair(m, interleave=None):
                c0, c1 = 2 * m, 2 * m + 1
                n0, n1 = 2 * c0 + 2, 2 * c1 + 2
                pso0 = ps_o.tile([128, 2 * F], f32, tag="ps_o", name="pso0")
                pso1 = ps_o.tile([128, 2 * F], f32, tag="ps_o", name="pso1")
                acc = accp.tile([128, 2 * F], bf16, tag="acc", name="acc")
                nc.gpsimd.memset(acc[:], 0.0)

                # the last pair runs its c1-only and diagonal tiles first so
                # the kernel ends on dense full-width unmasked work (short
                # exposed tail chain)
                if m == 3:
                    order = list(range(n0, n1)) + list(range(n0))
                else:
                    order = list(range(n1))
                c0_last = [ll for ll in order if ll < n0][-1]
                c1_last = order[-1]
                for idx, ll in enumerate(order):
                    if interleave is not None and idx == 2:
                        interleave()
                    both = ll < n0
                    pss = ps_a.tile([128, 2 * F], f32, tag="ps_a", name="pss")
                    pt = pTp.tile([128, 2 * F], bf16, tag="pT", name="pt")
                    regions = []   # (chunk, col base, live col start)
                    for ci, base in ([(c0, 0)] if both else []) + [(c1, F)]:
                        cs = 256 if ll == 2 * ci + 1 else 0
                        regions.append((ci, base, cs))
                    lo_all = regions[0][1] + regions[0][2]
                    hi = 2 * F
                    for ci, base, cs in regions:
                        for g in range(2):
                            nc.tensor.matmul(
                                pss[:, base + cs:base + F],
                                k2T[:, g, 128 * ll:128 * (ll + 1)],
                                xT[:, g, 512 * ci + cs:512 * (ci + 1)],
                                start=(g == 0), stop=(g == 1),
                                skip_group_check=True)
                    nc.scalar.activation(out=pt[:, lo_all:hi],
                                         in_=pss[:, lo_all:hi], func=Exp,
                                         bias=bqk_t[:, ll:ll + 1])
                    # diag masks: one 256-wide parity-encoded pattern
                    for ci, base, cs in regions:
                        if ll >= 2 * ci:
                            nc.vector.tensor_mul(
                                pt[:, base + cs:base + cs + 256],
                                pt[:, base + cs:base + cs + 256],
                                maskt[:])
                    # exp row-sum accumulation (bf16, DVE; acc pre-zeroed)
                    nc.vector.tensor_add(acc[:, lo_all:hi], acc[:, lo_all:hi],
                                         pt[:, lo_all:hi])
                    # P^T @ V accumulation
                    for ci, base, cs in regions:
                        pso = pso0 if ci == c0 else pso1
                        first = (ll == (0 if ci == c0 else order[0]))
                        last = (ll == (c0_last if ci == c0 else c1_last))
                        for ft in range(2):
                            nc.tensor.matmul(
                                pso[:, F * ft + cs:F * (ft + 1)],
                                v_sb[:, ll, 128 * ft:128 * (ft + 1)],
                                pt[:, base + cs:base + F],
                                start=first, stop=last,
                                skip_group_check=True)
                    if ll == c0_last:
                        po0 = pop.tile([128, 2 * F], bf16, tag="po", name="po0")
                        nc.vector.tensor_copy(out=po0[:], in_=pso0[:])
                        for ft in range(2):
                            nc.sync.dma_start(out=out_ext[c0, ft],
                                              in_=po0[:, F * ft:F * (ft + 1)])
                        nc.sync.dma_start(out=acc_ext[c0], in_=acc[:, 0:F])
                    if ll == c1_last:
                        po1 = pop.tile([128, 2 * F], bf16, tag="po", name="po1")
                        if m == 3:
                            # kernel tail: scalar is idle here; also spread
                            # the final DMAs across the three queues so their
                            # issue overheads overlap
                            nc.scalar.activation(out=po1[:], in_=pso1[:],
                                                 func=Copy)
                            nc.sync.dma_start(out=out_ext[c1, 0],
                                              in_=po1[:, 0:F])
                            nc.scalar.dma_start(out=out_ext[c1, 1],
                                                in_=po1[:, F:2 * F])
                            nc.gpsimd.dma_start(out=acc_ext[c1],
                                                in_=acc[:, F:2 * F])
                        else:
                            nc.vector.tensor_copy(out=po1[:], in_=pso1[:])
                            for ft in range(2):
                                nc.sync.dma_start(
                                    out=out_ext[c1, ft],
                                    in_=po1[:, F * ft:F * (ft + 1)])
                            nc.sync.dma_start(out=acc_ext[c1],
                                              in_=acc[:, F:2 * F])

            front_block(0)
            front_block(1)
            back_block(0)
            front_block(2)
            back_block(1)
            attn_pair(0, interleave=lambda: front_block(3))
            attn_pair(1, interleave=lambda: back_block(2))
            attn_pair(2, interleave=lambda: back_block(3))
            attn_pair(3)

    nc.compile()
    return nc


def _get_nc():
    if "nc" not in _COMPILED:
        _COMPILED["nc"] = _build()
    return _COMPILED["nc"]


def kernel(X, Z, mask, Wq, bq, Wk, bk, Wv, bv):
    import ml_dtypes
    bf16 = ml_dtypes.bfloat16

    X = np.asarray(X, dtype=np.float32)
    Z = np.asarray(Z, dtype=np.float32)
    mask_np = np.asarray(mask)
    Wq = np.asarray(Wq, dtype=np.float32)
    Wk = np.asarray(Wk, dtype=np.float32)
    Wv = np.asarray(Wv, dtype=np.float32)
    bq = np.asarray(bq, dtype=np.float32)
    bv = np.asarray(bv, dtype=np.float32)

    causal = bool(np.array_equal(
        mask_np != 0, np.tril(np.ones((S, S), dtype=bool))))
    if not causal:
        return _numpy_ref(X, Z, mask_np, Wq, bq, Wk, np.asarray(bk), Wv, bv)

    from concourse.bass_utils import run_bass_kernel_spmd

    nc = _get_nc()

    rsqE = np.float32(1.0 / np.sqrt(E))
    # wq ships in NATURAL [f, e] layout (stationary of the K2 fold-in),
    # carrying the 1/sqrt(E) score scale; wk/wv ship transposed [e, f].
    wq_n = np.ascontiguousarray((Wq * rsqE).astype(bf16)).reshape(2, 128, E)
    wk_t = np.ascontiguousarray(Wk.T.astype(bf16)).reshape(2, 128, E)
    wv_t = np.ascontiguousarray(Wv.T.astype(bf16)).reshape(2, 128, E)
    bqw_vec = (bq * rsqE) @ Wk          # e-vector: bqk = bqw . Z^T
    bqw = np.ascontiguousarray(bqw_vec.reshape(2, 128).T.astype(bf16))
    # bk is dropped: per-q-row constant in the scores -> softmax invariant.

    u = np.arange(256)[None, :]
    x = np.arange(128)[:, None]
    masks_par = [np.ascontiguousarray((u >= 128 * p + x).astype(bf16))
                 for p in range(2)]

    in_maps = []
    for c in range(8):
        b, p = c // 2, c % 2
        xt = np.ascontiguousarray(X[b].T.astype(bf16)).reshape(2, 128, S)
        zb = Z[b].reshape(S // 128, 128, E)[p::2].reshape(SK, E)
        zt = np.ascontiguousarray(zb.T.astype(bf16)).reshape(2, 128, SK)
        in_maps.append({
            "xt": xt, "zt": zt,
            "wq": wq_n, "wk": wk_t, "wv": wv_t,
            "bqw": bqw, "masks": masks_par[p],
        })

    res = run_bass_kernel_spmd(nc, in_maps, core_ids=list(range(8)))

    out = np.empty((B, S, E), dtype=np.float32)
    for b in range(B):
        r0, r1 = res.results[2 * b], res.results[2 * b + 1]
        num = r0["out"].astype(np.float32) + r1["out"].astype(np.float32)
        den = (r0["acc"].astype(np.float32).sum(axis=1)
               + r1["acc"].astype(np.float32).sum(axis=1))  # [NCH, F]
        ob = num.transpose(0, 3, 1, 2).reshape(S, E)
        out[b] = ob / den.reshape(S, 1) + bv
    return out


def _numpy_ref(X, Z, mask, Wq, bq, Wk, bk, Wv, bv):
    q = np.einsum("bse,fe->bsf", X, Wq) + bq
    k = np.einsum("bse,fe->bsf", Z, Wk) + bk
    v = np.einsum("bse,fe->bsf", Z, Wv) + bv
    s = np.einsum("bqe,bke->bqk", q, k) / np.sqrt(np.float32(X.shape[-1]))
    s = np.where(mask == 0, -np.inf, s)
    s = s - s.max(axis=-1, keepdims=True)
    p = np.exp(s)
    p /= p.sum(axis=-1, keepdims=True)
    return np.einsum("bqk,bke->bqe", p, v).astype(np.float32)


# revision 50
# speedup vs baseline: 1.1766x; 1.1766x over previous
"""Trainium2 Bass kernel: batched causal attention (B=4, S=4096, E=256, f32).

Sharding: 2 cores per batch element (4 pairs).  Within a pair, K/V rows are
split even/odd at 128-row tile granularity; both cores process all 4096 query
rows against their 2048 K/V rows.  The instruction stream is identical across
cores (pure SPMD, no collectives): each core ships its *unnormalized* partial
O^T (bf16) plus per-chunk exp-row-sum accumulators (bf16) to DRAM, and the
host merges the pair (add + normalize + transpose + bias) during unshard.

Device-side design notes:
  - Host pre-transposes and pre-casts X^T, Z^T, weights to bf16 -> no PE
    transposes and half the input DMA bytes.
  - The Q projection is folded into K ("K2" = Wq'^T K^T, with Wq' carrying
    the 1/sqrt(E) score scale): scores contract X^T directly against K2, so
    no Q-projection activations gate the attention pipeline.  The q-bias
    term bq.k varies only along k = the PSUM partition dim, so it rides the
    exp activation as a per-partition bias (bqk, via two 1-column matmuls
    per z-chunk).  The k-bias bk is dropped (softmax shift invariance).
  - Scores S^T[k,q] = K2(stationary) . X^T; chunk pairs (2m, 2m+1) run
    k-tiles in lockstep so one scalar-engine activation covers both score
    tiles.  The two single (c1-only) k-tiles run FIRST so the pair ends on
    dense full-width work.
  - Diagonal k-tiles are narrowed to the live 256..512 column range; the
    partially-masked leading 256-column block uses one parity-encoded mask.
  - Exp row-sums accumulate on DVE in bf16 into a memset-zeroed tile; the
    host does the final 128-partition reduction.
  - Projection "front" blocks (K/V matmuls + casts) are emitted well ahead
    of the "back" blocks (K2/bqk) that consume their DVE casts, which are
    themselves a pair ahead of the attention that consumes them.
"""

import numpy as np

B = 4
S = 4096
E = 256
SK = S // 2          # K/V rows per core
KT = SK // 128       # 16 local k-tiles
NCH = S // 512       # 8 q-chunks of 512
F = 512

_COMPILED = {}


def _build():
    import concourse.bass as bass
    import concourse.tile as tile
    from concourse import mybir, bacc

    from concourse.alu_op_type import AluOpType
    f32 = mybir.dt.float32
    bf16 = mybir.dt.bfloat16
    Exp = mybir.ActivationFunctionType.Exp
    Copy = mybir.ActivationFunctionType.Copy
    AluMult = AluOpType.mult
    AluAdd = AluOpType.add
    AluBypass = AluOpType.bypass

    nc = bacc.Bacc("TRN2", target_bir_lowering=False, debug=False,
                   enable_asserts=False, num_devices=1)

    xt_ext = nc.dram_tensor("xt", [2, 128, S], bf16, kind="ExternalInput")
    zt_ext = nc.dram_tensor("zt", [2, 128, SK], bf16, kind="ExternalInput")
    wq_ext = nc.dram_tensor("wq", [2, 128, E], bf16, kind="ExternalInput")
    wk_ext = nc.dram_tensor("wk", [2, 128, E], bf16, kind="ExternalInput")
    wv_ext = nc.dram_tensor("wv", [2, 128, E], bf16, kind="ExternalInput")
    # bqw = (bq/sqrt(E)) @ Wk : host-computed e-vector; bqk = bqw . Z^T
    bqw_ext = nc.dram_tensor("bqw", [128, 2], bf16, kind="ExternalInput")
    # diag mask: one parity-encoded 256-col pattern (keep iff u >= 128p + k)
    masks_ext = nc.dram_tensor("masks", [128, 256], bf16, kind="ExternalInput")
    out_ext = nc.dram_tensor("out", [NCH, 2, 128, F], bf16, kind="ExternalOutput")
    acc_ext = nc.dram_tensor("acc", [NCH, 128, F], bf16, kind="ExternalOutput")

    with tile.TileContext(nc) as tc:
        with tc.tile_pool(name="singles", bufs=1) as singles, \
             tc.tile_pool(name="pT", bufs=6) as pTp, \
             tc.tile_pool(name="accp", bufs=3) as accp, \
             tc.tile_pool(name="pop", bufs=4) as pop, \
             tc.tile_pool(name="dram", bufs=1, space="DRAM") as dram, \
             tc.tile_pool(name="ps_a", bufs=2, space="PSUM") as ps_a, \
             tc.tile_pool(name="ps_o", bufs=2, space="PSUM") as ps_o:

            # ---- persistent SBUF tensors ----------------------------------
            xT = singles.tile([128, 2, S], bf16, tag="xT")
            zT = singles.tile([128, 2, SK], bf16, tag="zT")
            kT = singles.tile([128, 2, SK], bf16, tag="kT")
            k2T = singles.tile([128, 2, SK], bf16, tag="k2T")
            v_sb = singles.tile([128, KT, E], bf16, tag="v_sb")
            wq_sb = singles.tile([128, 2, E], bf16, tag="wq_sb")
            wk_sb = singles.tile([128, 2, E], bf16, tag="wk_sb")
            wv_sb = singles.tile([128, 2, E], bf16, tag="wv_sb")
            bqw = singles.tile([128, 2], bf16, tag="bqw")
            bqk_row = singles.tile([1, SK], f32, tag="bqk_row")
            bqk_dram = dram.tile([SK], f32)
            bqk_t = singles.tile([128, KT], f32, tag="bqk_t")
            maskt = singles.tile([128, 256], bf16, tag="maskt")

            # ---- all input DMAs issued up front ---------------------------
            nc.scalar.dma_start(out=wk_sb[:],
                                in_=wk_ext.ap().rearrange("e p f -> p e f"))
            nc.scalar.dma_start(out=wv_sb[:],
                                in_=wv_ext.ap().rearrange("e p f -> p e f"))
            nc.gpsimd.dma_start(out=wq_sb[:],
                                in_=wq_ext.ap().rearrange("e p f -> p e f"))
            nc.gpsimd.dma_start(out=bqw[:], in_=bqw_ext[:])
            nc.gpsimd.dma_start(out=maskt[:], in_=masks_ext[:])
            # sync queue: first z half-slice split fine so the K projection
            # starts ASAP, then z/x interleaved in need order
            for et in range(2):
                nc.sync.dma_start(out=zT[:, et, 0:512],
                                  in_=zt_ext[et, :, 0:512])
            for et in range(2):
                nc.sync.dma_start(out=zT[:, et, 512:1024],
                                  in_=zt_ext[et, :, 512:1024])
            zx_order = [("x", 0), ("z", 1), ("x", 1), ("x", 2), ("x", 3)]
            for kind, h in zx_order:
                for et in range(2):
                    if kind == "z":
                        nc.sync.dma_start(
                            out=zT[:, et, 1024 * h:1024 * (h + 1)],
                            in_=zt_ext[et, :, 1024 * h:1024 * (h + 1)])
                    else:
                        nc.sync.dma_start(
                            out=xT[:, et, 1024 * h:1024 * (h + 1)],
                            in_=xt_ext[et, :, 1024 * h:1024 * (h + 1)])

            def front_block(sc):
                # K^T tile, exp-bias weights, and e^{bqk}-scaled natural-V
                # tile for z columns [512sc, 512(sc+1))
                psk = ps_a.tile([128, 2 * F], f32, tag="ps_a", name="psk")
                for g in range(2):
                    for et in range(2):
                        nc.tensor.matmul(psk[:, F * g:F * (g + 1)],
                                         wk_sb[:, et, 128 * g:128 * (g + 1)],
                                         zT[:, et, 512 * sc:512 * (sc + 1)],
                                         start=(et == 0), stop=(et == 1),
                                         skip_group_check=(g == 1))
                nc.vector.tensor_copy(out=kT[:, :, 512 * sc:512 * (sc + 1)],
                                      in_=psk[:])
                # bqk = bqw . Z^T, bounced through DRAM into [128, 4] layout
                psb = ps_a.tile([1, F], f32, tag="ps_a", name="psb",
                                padded_shape=[128, 2 * F])
                for et in range(2):
                    nc.tensor.matmul(psb[:], bqw[:, et:et + 1],
                                     zT[:, et, 512 * sc:512 * (sc + 1)],
                                     start=(et == 0), stop=(et == 1))
                nc.vector.tensor_copy(out=bqk_row[:, 512 * sc:512 * (sc + 1)],
                                      in_=psb[:])
                nc.gpsimd.dma_start(
                    out=bqk_dram[512 * sc:512 * (sc + 1)],
                    in_=bqk_row[0:1, 512 * sc:512 * (sc + 1)])
                nc.gpsimd.dma_start(
                    out=bqk_t[:, 4 * sc:4 * (sc + 1)],
                    in_=bqk_dram[512 * sc:512 * (sc + 1)].rearrange(
                        "(l p) -> p l", p=128))
                psv = ps_a.tile([128, 4, E], f32, tag="ps_a", name="psv",
                                padded_shape=[128, 4, E])
                for t in range(4):
                    base = 512 * sc + 128 * t
                    for et in range(2):
                        nc.tensor.matmul(psv[:, t, :],
                                         zT[:, et, base:base + 128],
                                         wv_sb[:, et, :],
                                         start=(et == 0), stop=(et == 1),
                                         skip_group_check=(t > 0))
                nc.vector.tensor_copy(out=v_sb[:, 4 * sc:4 * sc + 4, :],
                                      in_=psv[:])

            def back_block(sc):
                # K2 = Wq'^T K^T (scores then contract X directly)
                psk2 = ps_a.tile([128, 2 * F], f32, tag="ps_a", name="psk2")
                for g in range(2):
                    for ft in range(2):
                        nc.tensor.matmul(psk2[:, F * g:F * (g + 1)],
                                         wq_sb[:, ft, 128 * g:128 * (g + 1)],
                                         kT[:, ft, 512 * sc:512 * (sc + 1)],
                                         start=(ft == 0), stop=(ft == 1),
                                         skip_group_check=(g == 1))
                nc.vector.tensor_copy(out=k2T[:, :, 512 * sc:512 * (sc + 1)],
                                      in_=psk2[:])

            def attn_pair(m, interleave=None):
                c0, c1 = 2 * m, 2 * m + 1
                n0, n1 = 2 * c0 + 2, 2 * c1 + 2
                pso0 = ps_o.tile([128, 2 * F], f32, tag="ps_o", name="pso0")
                pso1 = ps_o.tile([128, 2 * F], f32, tag="ps_o", name="pso1")
                acc = accp.tile([128, 2 * F], bf16, tag="acc", name="acc")
                nc.gpsimd.memset(acc[:], 0.0)

                # the last pair runs its c1-only and diagonal tiles first so
                # the kernel ends on dense full-width unmasked work (short
                # exposed tail chain)
                if m == 3:
                    order = list(range(n0, n1)) + list(range(n0))
                else:
                    order = list(range(n1))
                c0_last = [ll for ll in order if ll < n0][-1]
                c1_last = order[-1]
                for idx, ll in enumerate(order):
                    if interleave is not None and idx == 2:
                        interleave()
                    both = ll < n0
                    pss = ps_a.tile([128, 2 * F], f32, tag="ps_a", name="pss")
                    pt = pTp.tile([128, 2 * F], bf16, tag="pT", name="pt")
                    regions = []   # (chunk, col base, live col start)
                    for ci, base in ([(c0, 0)] if both else []) + [(c1, F)]:
                        cs = 256 if ll == 2 * ci + 1 else 0
                        regions.append((ci, base, cs))
                    lo_all = regions[0][1] + regions[0][2]
                    hi = 2 * F
                    for ci, base, cs in regions:
                        for g in range(2):
                            nc.tensor.matmul(
                                pss[:, base + cs:base + F],
                                k2T[:, g, 128 * ll:128 * (ll + 1)],
                                xT[:, g, 512 * ci + cs:512 * (ci + 1)],
                                start=(g == 0), stop=(g == 1),
                                skip_group_check=True)
                    nc.scalar.activation(out=pt[:, lo_all:hi],
                                         in_=pss[:, lo_all:hi], func=Exp,
                                         bias=bqk_t[:, ll:ll + 1])
                    # diag masks: one 256-wide parity-encoded pattern
                    for ci, base, cs in regions:
                        if ll >= 2 * ci:
                            nc.vector.tensor_mul(
                                pt[:, base + cs:base + cs + 256],
                                pt[:, base + cs:base + cs + 256],
                                maskt[:])
                    # exp row-sum accumulation (bf16, DVE; acc pre-zeroed)
                    nc.vector.tensor_add(acc[:, lo_all:hi], acc[:, lo_all:hi],
                                         pt[:, lo_all:hi])
                    # P^T @ V accumulation
                    for ci, base, cs in regions:
                        pso = pso0 if ci == c0 else pso1
                        first = (ll == (0 if ci == c0 else order[0]))
                        last = (ll == (c0_last if ci == c0 else c1_last))
                        for ft in range(2):
                            nc.tensor.matmul(
                                pso[:, F * ft + cs:F * (ft + 1)],
                                v_sb[:, ll, 128 * ft:128 * (ft + 1)],
                                pt[:, base + cs:base + F],
                                start=first, stop=last,
                                skip_group_check=True)
                    if ll == c0_last:
                        po0 = pop.tile([128, 2 * F], bf16, tag="po", name="po0")
                        nc.vector.tensor_copy(out=po0[:], in_=pso0[:])
                        for ft in range(2):
                            nc.sync.dma_start(out=out_ext[c0, ft],
                                              in_=po0[:, F * ft:F * (ft + 1)])
                        nc.sync.dma_start(out=acc_ext[c0], in_=acc[:, 0:F])
                    if ll == c1_last:
                        po1 = pop.tile([128, 2 * F], bf16, tag="po", name="po1")
                        if m == 3:
                            # kernel tail: scalar is idle here; also spread
                            # the final DMAs across the three queues so their
                            # issue overheads overlap
                            nc.scalar.activation(out=po1[:], in_=pso1[:],
                                                 func=Copy)
                            nc.sync.dma_start(out=out_ext[c1, 0],
                                              in_=po1[:, 0:F])
                            nc.scalar.dma_start(out=out_ext[c1, 1],
                                                in_=po1[:, F:2 * F])
                            nc.gpsimd.dma_start(out=acc_ext[c1],
                                                in_=acc[:, F:2 * F])
                        else:
                            nc.vector.tensor_copy(out=po1[:], in_=pso1[:])
                            for ft in range(2):
                                nc.sync.dma_start(
                                    out=out_ext[c1, ft],
                                    in_=po1[:, F * ft:F * (ft + 1)])
                            nc.sync.dma_start(out=acc_ext[c1],
                                              in_=acc[:, F:2 * F])

            front_block(0)
            front_block(1)
            back_block(0)
            front_block(2)
            back_block(1)
            attn_pair(0, interleave=lambda: front_block(3))
            attn_pair(1, interleave=lambda: back_block(2))
            attn_pair(2, interleave=lambda: back_block(3))
            attn_pair(3)

    nc.compile()
    return nc


def _get_nc():
    if "nc" not in _COMPILED:
        _COMPILED["nc"] = _build()
    return _COMPILED["nc"]


def kernel(X, Z, mask, Wq, bq, Wk, bk, Wv, bv):
    import ml_dtypes
    bf16 = ml_dtypes.bfloat16

    X = np.asarray(X, dtype=np.float32)
    Z = np.asarray(Z, dtype=np.float32)
    mask_np = np.asarray(mask)
    Wq = np.asarray(Wq, dtype=np.float32)
    Wk = np.asarray(Wk, dtype=np.float32)
    Wv = np.asarray(Wv, dtype=np.float32)
    bq = np.asarray(bq, dtype=np.float32)
    bv = np.asarray(bv, dtype=np.float32)

    causal = bool(np.array_equal(
        mask_np != 0, np.tril(np.ones((S, S), dtype=bool))))
    if not causal:
        return _numpy_ref(X, Z, mask_np, Wq, bq, Wk, np.asarray(bk), Wv, bv)

    from concourse.bass_utils import run_bass_kernel_spmd

    nc = _get_nc()

    rsqE = np.float32(1.0 / np.sqrt(E))
    # wq ships in NATURAL [f, e] layout (stationary of the K2 fold-in),
    # carrying the 1/sqrt(E) score scale; wk/wv ship transposed [e, f].
    wq_n = np.ascontiguousarray((Wq * rsqE).astype(bf16)).reshape(2, 128, E)
    wk_t = np.ascontiguousarray(Wk.T.astype(bf16)).reshape(2, 128, E)
    wv_t = np.ascontiguousarray(Wv.T.astype(bf16)).reshape(2, 128, E)
    bqw_vec = (bq * rsqE) @ Wk          # e-vector: bqk = bqw . Z^T
    bqw = np.ascontiguousarray(bqw_vec.reshape(2, 128).T.astype(bf16))
    # bk is dropped: per-q-row constant in the scores -> softmax invariant.

    u = np.arange(256)[None, :]
    x = np.arange(128)[:, None]
    masks_par = [np.ascontiguousarray((u >= 128 * p + x).astype(bf16))
                 for p in range(2)]

    in_maps = []
    for c in range(8):
        b, p = c // 2, c % 2
        xt = np.ascontiguousarray(X[b].T.astype(bf16)).reshape(2, 128, S)
        zb = Z[b].reshape(S // 128, 128, E)[p::2].reshape(SK, E)
        zt = np.ascontiguousarray(zb.T.astype(bf16)).reshape(2, 128, SK)
        in_maps.append({
            "xt": xt, "zt": zt,
            "wq": wq_n, "wk": wk_t, "wv": wv_t,
            "bqw": bqw, "masks": masks_par[p],
        })

    res = run_bass_kernel_spmd(nc, in_maps, core_ids=list(range(8)))

    out = np.empty((B, S, E), dtype=np.float32)
    for b in range(B):
        r0, r1 = res.results[2 * b], res.results[2 * b + 1]
        num = r0["out"].astype(np.float32) + r1["out"].astype(np.float32)
        den = (r0["acc"].astype(np.float32).sum(axis=1)
               + r1["acc"].astype(np.float32).sum(axis=1))  # [NCH, F]
        ob = num.transpose(0, 3, 1, 2).reshape(S, E)
        out[b] = ob / den.reshape(S, 1) + bv
    return out


def _numpy_ref(X, Z, mask, Wq, bq, Wk, bk, Wv, bv):
    q = np.einsum("bse,fe->bsf", X, Wq) + bq
    k = np.einsum("bse,fe->bsf", Z, Wk) + bk
    v = np.einsum("bse,fe->bsf", Z, Wv) + bv
    s = np.einsum("bqe,bke->bqk", q, k) / np.sqrt(np.float32(X.shape[-1]))
    s = np.where(mask == 0, -np.inf, s)
    s = s - s.max(axis=-1, keepdims=True)
    p = np.exp(s)
    p /= p.sum(axis=-1, keepdims=True)
    return np.einsum("bqk,bke->bqe", p, v).astype(np.float32)


# revision 52
# speedup vs baseline: 1.1826x; 1.0051x over previous
"""Trainium2 Bass kernel: batched causal attention (B=4, S=4096, E=256, f32).

Sharding: 2 cores per batch element (4 pairs).  Within a pair, K/V rows are
split even/odd at 128-row tile granularity; both cores process all 4096 query
rows against their 2048 K/V rows.  The instruction stream is identical across
cores (pure SPMD, no collectives): each core ships its *unnormalized* partial
O^T (bf16) plus per-chunk exp-row-sum accumulators (bf16) to DRAM, and the
host merges the pair (add + normalize + transpose + bias) during unshard.

Device-side design notes:
  - Host pre-transposes and pre-casts X^T, Z^T, weights to bf16 -> no PE
    transposes and half the input DMA bytes.
  - The Q projection is folded into K ("K2" = Wq'^T K^T, with Wq' carrying
    the 1/sqrt(E) score scale): scores contract X^T directly against K2, so
    no Q-projection activations gate the attention pipeline.  The q-bias
    term bq.k varies only along k = the PSUM partition dim, so it rides the
    exp activation as a per-partition bias (bqk, via two 1-column matmuls
    per z-chunk).  The k-bias bk is dropped (softmax shift invariance).
  - Scores S^T[k,q] = K2(stationary) . X^T; chunk pairs (2m, 2m+1) run
    k-tiles in lockstep so one scalar-engine activation covers both score
    tiles.  The final pair runs its two c1-only k-tiles first so the kernel
    ends on dense full-width work (short exposed tail chain).
  - Diagonal k-tiles are narrowed to the live 256..512 column range; the
    partially-masked leading 256-column block uses one parity-encoded mask.
  - Exp row-sums accumulate on DVE in bf16 into a memset-zeroed tile; the
    host does the final 128-partition reduction.
  - Projection "front" blocks (K/V matmuls + casts) are emitted well ahead
    of the "back" blocks (K2/bqk) that consume their DVE casts, which are
    themselves a pair ahead of the attention that consumes them.
"""

import numpy as np

B = 4
S = 4096
E = 256
SK = S // 2          # K/V rows per core
KT = SK // 128       # 16 local k-tiles
NCH = S // 512       # 8 q-chunks of 512
F = 512

_COMPILED = {}


def _build():
    import concourse.tile as tile
    from concourse import mybir, bacc

    f32 = mybir.dt.float32
    bf16 = mybir.dt.bfloat16
    Exp = mybir.ActivationFunctionType.Exp
    Copy = mybir.ActivationFunctionType.Copy

    nc = bacc.Bacc("TRN2", target_bir_lowering=False, debug=False,
                   enable_asserts=False, num_devices=1)

    xt_ext = nc.dram_tensor("xt", [2, 128, S], bf16, kind="ExternalInput")
    zt_ext = nc.dram_tensor("zt", [2, 128, SK], bf16, kind="ExternalInput")
    wq_ext = nc.dram_tensor("wq", [2, 128, E], bf16, kind="ExternalInput")
    wk_ext = nc.dram_tensor("wk", [2, 128, E], bf16, kind="ExternalInput")
    wv_ext = nc.dram_tensor("wv", [2, 128, E], bf16, kind="ExternalInput")
    # bqw = (bq/sqrt(E)) @ Wk : host-computed e-vector; bqk = bqw . Z^T
    bqw_ext = nc.dram_tensor("bqw", [128, 2], bf16, kind="ExternalInput")
    # diag mask: one parity-encoded 256-col pattern (keep iff u >= 128p + k)
    masks_ext = nc.dram_tensor("masks", [128, 256], bf16, kind="ExternalInput")
    out_ext = nc.dram_tensor("out", [NCH, 2, 128, F], bf16, kind="ExternalOutput")
    acc_ext = nc.dram_tensor("acc", [NCH, 128, F], bf16, kind="ExternalOutput")

    with tile.TileContext(nc) as tc:
        with tc.tile_pool(name="singles", bufs=1) as singles, \
             tc.tile_pool(name="pT", bufs=6) as pTp, \
             tc.tile_pool(name="accp", bufs=3) as accp, \
             tc.tile_pool(name="pop", bufs=4) as pop, \
             tc.tile_pool(name="dram", bufs=1, space="DRAM") as dram, \
             tc.tile_pool(name="ps_a", bufs=2, space="PSUM") as ps_a, \
             tc.tile_pool(name="ps_o", bufs=2, space="PSUM") as ps_o:

            # ---- persistent SBUF tensors ----------------------------------
            xT = singles.tile([128, 2, S], bf16, tag="xT")
            zT = singles.tile([128, 2, SK], bf16, tag="zT")
            kT = singles.tile([128, 2, SK], bf16, tag="kT")
            k2T = singles.tile([128, 2, SK], bf16, tag="k2T")
            v_sb = singles.tile([128, KT, E], bf16, tag="v_sb")
            wq_sb = singles.tile([128, 2, E], bf16, tag="wq_sb")
            wk_sb = singles.tile([128, 2, E], bf16, tag="wk_sb")
            wv_sb = singles.tile([128, 2, E], bf16, tag="wv_sb")
            bqw = singles.tile([128, 2], bf16, tag="bqw")
            bqk_row = singles.tile([1, SK], f32, tag="bqk_row")
            bqk_dram = dram.tile([SK], f32)
            bqk_t = singles.tile([128, KT], f32, tag="bqk_t")
            maskt = singles.tile([128, 256], bf16, tag="maskt")

            # ---- all input DMAs issued up front ---------------------------
            nc.scalar.dma_start(out=wk_sb[:],
                                in_=wk_ext.ap().rearrange("e p f -> p e f"))
            nc.scalar.dma_start(out=wv_sb[:],
                                in_=wv_ext.ap().rearrange("e p f -> p e f"))
            nc.gpsimd.dma_start(out=wq_sb[:],
                                in_=wq_ext.ap().rearrange("e p f -> p e f"))
            nc.gpsimd.dma_start(out=bqw[:], in_=bqw_ext[:])
            nc.gpsimd.dma_start(out=maskt[:], in_=masks_ext[:])
            # sync queue: first z half-slice split fine so the K projection
            # starts ASAP, then z/x interleaved in need order
            for et in range(2):
                nc.sync.dma_start(out=zT[:, et, 0:512],
                                  in_=zt_ext[et, :, 0:512])
            for et in range(2):
                nc.sync.dma_start(out=zT[:, et, 512:1024],
                                  in_=zt_ext[et, :, 512:1024])
            zx_order = [("x", 0), ("z", 1), ("x", 1), ("x", 2), ("x", 3)]
            for kind, h in zx_order:
                for et in range(2):
                    if kind == "z":
                        nc.sync.dma_start(
                            out=zT[:, et, 1024 * h:1024 * (h + 1)],
                            in_=zt_ext[et, :, 1024 * h:1024 * (h + 1)])
                    else:
                        nc.sync.dma_start(
                            out=xT[:, et, 1024 * h:1024 * (h + 1)],
                            in_=xt_ext[et, :, 1024 * h:1024 * (h + 1)])

            def front_block(sc):
                # K^T tile, exp-bias weights, and e^{bqk}-scaled natural-V
                # tile for z columns [512sc, 512(sc+1))
                psk = ps_a.tile([128, 2 * F], f32, tag="ps_a", name="psk")
                for g in range(2):
                    for et in range(2):
                        nc.tensor.matmul(psk[:, F * g:F * (g + 1)],
                                         wk_sb[:, et, 128 * g:128 * (g + 1)],
                                         zT[:, et, 512 * sc:512 * (sc + 1)],
                                         start=(et == 0), stop=(et == 1),
                                         skip_group_check=(g == 1))
                nc.vector.tensor_copy(out=kT[:, :, 512 * sc:512 * (sc + 1)],
                                      in_=psk[:])
                # bqk = bqw . Z^T, bounced through DRAM into [128, 4] layout
                psb = ps_a.tile([1, F], f32, tag="ps_a", name="psb",
                                padded_shape=[128, 2 * F])
                for et in range(2):
                    nc.tensor.matmul(psb[:], bqw[:, et:et + 1],
                                     zT[:, et, 512 * sc:512 * (sc + 1)],
                                     start=(et == 0), stop=(et == 1))
                nc.vector.tensor_copy(out=bqk_row[:, 512 * sc:512 * (sc + 1)],
                                      in_=psb[:])
                nc.gpsimd.dma_start(
                    out=bqk_dram[512 * sc:512 * (sc + 1)],
                    in_=bqk_row[0:1, 512 * sc:512 * (sc + 1)])
                nc.gpsimd.dma_start(
                    out=bqk_t[:, 4 * sc:4 * (sc + 1)],
                    in_=bqk_dram[512 * sc:512 * (sc + 1)].rearrange(
                        "(l p) -> p l", p=128))
                psv = ps_a.tile([128, 4, E], f32, tag="ps_a", name="psv",
                                padded_shape=[128, 4, E])
                for t in range(4):
                    base = 512 * sc + 128 * t
                    for et in range(2):
                        nc.tensor.matmul(psv[:, t, :],
                                         zT[:, et, base:base + 128],
                                         wv_sb[:, et, :],
                                         start=(et == 0), stop=(et == 1),
                                         skip_group_check=(t > 0))
                nc.vector.tensor_copy(out=v_sb[:, 4 * sc:4 * sc + 4, :],
                                      in_=psv[:])

            def back_block(sc):
                # K2 = Wq'^T K^T (scores then contract X directly)
                psk2 = ps_a.tile([128, 2 * F], f32, tag="ps_a", name="psk2")
                for g in range(2):
                    for ft in range(2):
                        nc.tensor.matmul(psk2[:, F * g:F * (g + 1)],
                                         wq_sb[:, ft, 128 * g:128 * (g + 1)],
                                         kT[:, ft, 512 * sc:512 * (sc + 1)],
                                         start=(ft == 0), stop=(ft == 1),
                                         skip_group_check=(g == 1))
                nc.vector.tensor_copy(out=k2T[:, :, 512 * sc:512 * (sc + 1)],
                                      in_=psk2[:])

            def attn_pair(m, interleave=None):
                c0, c1 = 2 * m, 2 * m + 1
                n0, n1 = 2 * c0 + 2, 2 * c1 + 2
                pso0 = ps_o.tile([128, 2 * F], f32, tag="ps_o", name="pso0")
                pso1 = ps_o.tile([128, 2 * F], f32, tag="ps_o", name="pso1")
                acc = accp.tile([128, 2 * F], bf16, tag="acc", name="acc")
                nc.gpsimd.memset(acc[:], 0.0)

                # the last pair runs its c1-only and diagonal tiles first so
                # the kernel ends on dense full-width unmasked work (short
                # exposed tail chain)
                if m == 3:
                    order = list(range(n0, n1)) + list(range(n0))
                else:
                    order = list(range(n1))
                c0_last = [ll for ll in order if ll < n0][-1]
                c1_last = order[-1]
                for idx, ll in enumerate(order):
                    if interleave is not None and idx == 2:
                        interleave()
                    both = ll < n0
                    pss = ps_a.tile([128, 2 * F], f32, tag="ps_a", name="pss")
                    pt = pTp.tile([128, 2 * F], bf16, tag="pT", name="pt")
                    regions = []   # (chunk, col base, live col start)
                    for ci, base in ([(c0, 0)] if both else []) + [(c1, F)]:
                        cs = 256 if ll == 2 * ci + 1 else 0
                        regions.append((ci, base, cs))
                    lo_all = regions[0][1] + regions[0][2]
                    hi = 2 * F
                    for ci, base, cs in regions:
                        for g in range(2):
                            nc.tensor.matmul(
                                pss[:, base + cs:base + F],
                                k2T[:, g, 128 * ll:128 * (ll + 1)],
                                xT[:, g, 512 * ci + cs:512 * (ci + 1)],
                                start=(g == 0), stop=(g == 1),
                                skip_group_check=True)
                    nc.scalar.activation(out=pt[:, lo_all:hi],
                                         in_=pss[:, lo_all:hi], func=Exp,
                                         bias=bqk_t[:, ll:ll + 1])
                    # diag masks: one 256-wide parity-encoded pattern
                    for ci, base, cs in regions:
                        if ll >= 2 * ci:
                            nc.vector.tensor_mul(
                                pt[:, base + cs:base + cs + 256],
                                pt[:, base + cs:base + cs + 256],
                                maskt[:])
                    # exp row-sum accumulation (bf16, DVE; acc pre-zeroed)
                    nc.vector.tensor_add(acc[:, lo_all:hi], acc[:, lo_all:hi],
                                         pt[:, lo_all:hi])
                    # P^T @ V accumulation
                    for ci, base, cs in regions:
                        pso = pso0 if ci == c0 else pso1
                        first = (ll == (0 if ci == c0 else order[0]))
                        last = (ll == (c0_last if ci == c0 else c1_last))
                        for ft in range(2):
                            nc.tensor.matmul(
                                pso[:, F * ft + cs:F * (ft + 1)],
                                v_sb[:, ll, 128 * ft:128 * (ft + 1)],
                                pt[:, base + cs:base + F],
                                start=first, stop=last,
                                skip_group_check=True)
                    if ll == c0_last:
                        po0 = pop.tile([128, 2 * F], bf16, tag="po", name="po0")
                        nc.vector.tensor_copy(out=po0[:], in_=pso0[:])
                        for ft in range(2):
                            nc.sync.dma_start(out=out_ext[c0, ft],
                                              in_=po0[:, F * ft:F * (ft + 1)])
                        nc.sync.dma_start(out=acc_ext[c0], in_=acc[:, 0:F])
                    if ll == c1_last:
                        po1 = pop.tile([128, 2 * F], bf16, tag="po", name="po1")
                        if m == 3:
                            # kernel tail: scalar is idle here; also spread
                            # the final DMAs across the three queues so their
                            # issue overheads overlap
                            nc.scalar.activation(out=po1[:], in_=pso1[:],
                                                 func=Copy)
                            nc.sync.dma_start(out=out_ext[c1, 0],
                                              in_=po1[:, 0:F])
                            nc.scalar.dma_start(out=out_ext[c1, 1],
                                                in_=po1[:, F:2 * F])
                            nc.gpsimd.dma_start(out=acc_ext[c1],
                                                in_=acc[:, F:2 * F])
                        else:
                            nc.vector.tensor_copy(out=po1[:], in_=pso1[:])
                            for ft in range(2):
                                nc.sync.dma_start(
                                    out=out_ext[c1, ft],
                                    in_=po1[:, F * ft:F * (ft + 1)])
                            nc.sync.dma_start(out=acc_ext[c1],
                                              in_=acc[:, F:2 * F])

            front_block(0)
            front_block(1)
            back_block(0)
            front_block(2)
            back_block(1)
            attn_pair(0, interleave=lambda: front_block(3))
            attn_pair(1, interleave=lambda: back_block(2))
            attn_pair(2, interleave=lambda: back_block(3))
            attn_pair(3)

    nc.compile()
    return nc


def _get_nc():
    if "nc" not in _COMPILED:
        _COMPILED["nc"] = _build()
    return _COMPILED["nc"]


def kernel(X, Z, mask, Wq, bq, Wk, bk, Wv, bv):
    import ml_dtypes
    bf16 = ml_dtypes.bfloat16

    X = np.asarray(X, dtype=np.float32)
    Z = np.asarray(Z, dtype=np.float32)
    mask_np = np.asarray(mask)
    Wq = np.asarray(Wq, dtype=np.float32)
    Wk = np.asarray(Wk, dtype=np.float32)
    Wv = np.asarray(Wv, dtype=np.float32)
    bq = np.asarray(bq, dtype=np.float32)
    bv = np.asarray(bv, dtype=np.float32)

    causal = bool(np.array_equal(
        mask_np != 0, np.tril(np.ones((S, S), dtype=bool))))
    if not causal:
        return _numpy_ref(X, Z, mask_np, Wq, bq, Wk, np.asarray(bk), Wv, bv)

    from concourse.bass_utils import run_bass_kernel_spmd

    nc = _get_nc()

    rsqE = np.float32(1.0 / np.sqrt(E))
    # wq ships in NATURAL [f, e] layout (stationary of the K2 fold-in),
    # carrying the 1/sqrt(E) score scale; wk/wv ship transposed [e, f].
    wq_n = np.ascontiguousarray((Wq * rsqE).astype(bf16)).reshape(2, 128, E)
    wk_t = np.ascontiguousarray(Wk.T.astype(bf16)).reshape(2, 128, E)
    wv_t = np.ascontiguousarray(Wv.T.astype(bf16)).reshape(2, 128, E)
    bqw_vec = (bq * rsqE) @ Wk          # e-vector: bqk = bqw . Z^T
    bqw = np.ascontiguousarray(bqw_vec.reshape(2, 128).T.astype(bf16))
    # bk is dropped: per-q-row constant in the scores -> softmax invariant.

    u = np.arange(256)[None, :]
    x = np.arange(128)[:, None]
    masks_par = [np.ascontiguousarray((u >= 128 * p + x).astype(bf16))
                 for p in range(2)]

    in_maps = []
    for c in range(8):
        b, p = c // 2, c % 2
        xt = np.ascontiguousarray(X[b].T.astype(bf16)).reshape(2, 128, S)
        zb = Z[b].reshape(S // 128, 128, E)[p::2].reshape(SK, E)
        zt = np.ascontiguousarray(zb.T.astype(bf16)).reshape(2, 128, SK)
        in_maps.append({
            "xt": xt, "zt": zt,
            "wq": wq_n, "wk": wk_t, "wv": wv_t,
            "bqw": bqw, "masks": masks_par[p],
        })

    res = run_bass_kernel_spmd(nc, in_maps, core_ids=list(range(8)))

    out = np.empty((B, S, E), dtype=np.float32)
    for b in range(B):
        r0, r1 = res.results[2 * b], res.results[2 * b + 1]
        num = r0["out"].astype(np.float32) + r1["out"].astype(np.float32)
        den = (r0["acc"].astype(np.float32).sum(axis=1)
               + r1["acc"].astype(np.float32).sum(axis=1))  # [NCH, F]
        ob = num.transpose(0, 3, 1, 2).reshape(S, E)
        out[b] = ob / den.reshape(S, 1) + bv
    return out


def _numpy_ref(X, Z, mask, Wq, bq, Wk, bk, Wv, bv):
    q = np.einsum("bse,fe->bsf", X, Wq) + bq
    k = np.einsum("bse,fe->bsf", Z, Wk) + bk
    v = np.einsum("bse,fe->bsf", Z, Wv) + bv
    s = np.einsum("bqe,bke->bqk", q, k) / np.sqrt(np.float32(X.shape[-1]))
    s = np.where(mask == 0, -np.inf, s)
    s = s - s.max(axis=-1, keepdims=True)
    p = np.exp(s)
    p /= p.sum(axis=-1, keepdims=True)
    return np.einsum("bqk,bke->bqe", p, v).astype(np.float32)


# revision 54
# speedup vs baseline: 1.1841x; 1.0013x over previous
"""Trainium2 Bass kernel: batched causal attention (B=4, S=4096, E=256, f32).

Sharding: 2 cores per batch element (4 pairs).  Within a pair, K/V rows are
split even/odd at 128-row tile granularity; both cores process all 4096 query
rows against their 2048 K/V rows.  The instruction stream is identical across
cores (pure SPMD, no collectives): each core ships its *unnormalized* partial
O^T (bf16) plus per-chunk exp-row-sum accumulators (bf16) to DRAM, and the
host merges the pair (add + normalize + transpose + bias) during unshard.

Device-side design notes:
  - Host pre-transposes and pre-casts X^T, Z^T, weights to bf16 -> no PE
    transposes and half the input DMA bytes.
  - The Q projection is folded into K ("K2" = Wq'^T K^T, with Wq' carrying
    the 1/sqrt(E) score scale): scores contract X^T directly against K2, so
    no Q-projection activations gate the attention pipeline.  The q-bias
    term bq.k varies only along k = the PSUM partition dim, so it rides the
    exp activation as a per-partition bias (bqk, via two 1-column matmuls
    per z-chunk).  The k-bias bk is dropped (softmax shift invariance).
  - Scores S^T[k,q] = K2(stationary) . X^T; chunk pairs (2m, 2m+1) run
    k-tiles in lockstep so one scalar-engine activation covers both score
    tiles.  The final pair runs its two c1-only k-tiles first so the kernel
    ends on dense full-width work (short exposed tail chain).
  - Diagonal k-tiles are narrowed to the live 256..512 column range; the
    partially-masked leading 256-column block uses one parity-encoded mask.
  - Exp row-sums accumulate on DVE in bf16 into a memset-zeroed tile; the
    host does the final 128-partition reduction.
  - Projection "front" blocks (K/V matmuls + casts) are emitted well ahead
    of the "back" blocks (K2/bqk) that consume their DVE casts, which are
    themselves a pair ahead of the attention that consumes them.
"""

import numpy as np

B = 4
S = 4096
E = 256
SK = S // 2          # K/V rows per core
KT = SK // 128       # 16 local k-tiles
NCH = S // 512       # 8 q-chunks of 512
F = 512

_COMPILED = {}


def _build():
    import concourse.tile as tile
    from concourse import mybir, bacc

    f32 = mybir.dt.float32
    bf16 = mybir.dt.bfloat16
    Exp = mybir.ActivationFunctionType.Exp
    Copy = mybir.ActivationFunctionType.Copy

    nc = bacc.Bacc("TRN2", target_bir_lowering=False, debug=False,
                   enable_asserts=False, num_devices=1)

    xt_ext = nc.dram_tensor("xt", [2, 128, S], bf16, kind="ExternalInput")
    zt_ext = nc.dram_tensor("zt", [2, 128, SK], bf16, kind="ExternalInput")
    wq_ext = nc.dram_tensor("wq", [2, 128, E], bf16, kind="ExternalInput")
    wk_ext = nc.dram_tensor("wk", [2, 128, E], bf16, kind="ExternalInput")
    wv_ext = nc.dram_tensor("wv", [2, 128, E], bf16, kind="ExternalInput")
    # bqw = (bq/sqrt(E)) @ Wk : host-computed e-vector; bqk = bqw . Z^T
    bqw_ext = nc.dram_tensor("bqw", [128, 2], bf16, kind="ExternalInput")
    # diag mask: one parity-encoded 256-col pattern (keep iff u >= 128p + k)
    masks_ext = nc.dram_tensor("masks", [128, 256], bf16, kind="ExternalInput")
    out_ext = nc.dram_tensor("out", [NCH, 2, 128, F], bf16, kind="ExternalOutput")
    acc_ext = nc.dram_tensor("acc", [NCH, 128, F], bf16, kind="ExternalOutput")

    with tile.TileContext(nc) as tc:
        with tc.tile_pool(name="singles", bufs=1) as singles, \
             tc.tile_pool(name="pT", bufs=6) as pTp, \
             tc.tile_pool(name="accp", bufs=3) as accp, \
             tc.tile_pool(name="pop", bufs=4) as pop, \
             tc.tile_pool(name="dram", bufs=1, space="DRAM") as dram, \
             tc.tile_pool(name="ps_a", bufs=2, space="PSUM") as ps_a, \
             tc.tile_pool(name="ps_o", bufs=2, space="PSUM") as ps_o:

            # ---- persistent SBUF tensors ----------------------------------
            xT = singles.tile([128, 2, S], bf16, tag="xT")
            zT = singles.tile([128, 2, SK], bf16, tag="zT")
            kT = singles.tile([128, 2, SK], bf16, tag="kT")
            k2T = singles.tile([128, 2, SK], bf16, tag="k2T")
            v_sb = singles.tile([128, KT, E], bf16, tag="v_sb")
            wq_sb = singles.tile([128, 2, E], bf16, tag="wq_sb")
            wk_sb = singles.tile([128, 2, E], bf16, tag="wk_sb")
            wv_sb = singles.tile([128, 2, E], bf16, tag="wv_sb")
            bqw = singles.tile([128, 2], bf16, tag="bqw")
            bqk_row = singles.tile([1, SK], f32, tag="bqk_row")
            bqk_dram = dram.tile([SK], f32)
            bqk_t = singles.tile([128, KT], f32, tag="bqk_t")
            maskt = singles.tile([128, 256], bf16, tag="maskt")

            # ---- all input DMAs issued up front ---------------------------
            nc.scalar.dma_start(out=wk_sb[:],
                                in_=wk_ext.ap().rearrange("e p f -> p e f"))
            nc.scalar.dma_start(out=wv_sb[:],
                                in_=wv_ext.ap().rearrange("e p f -> p e f"))
            nc.gpsimd.dma_start(out=wq_sb[:],
                                in_=wq_ext.ap().rearrange("e p f -> p e f"))
            nc.gpsimd.dma_start(out=bqw[:], in_=bqw_ext[:])
            nc.gpsimd.dma_start(out=maskt[:], in_=masks_ext[:])
            # sync queue: first z half-slice split fine so the K projection
            # starts ASAP, then z/x interleaved in need order
            for et in range(2):
                nc.sync.dma_start(out=zT[:, et, 0:512],
                                  in_=zt_ext[et, :, 0:512])
            for et in range(2):
                nc.sync.dma_start(out=zT[:, et, 512:1024],
                                  in_=zt_ext[et, :, 512:1024])
            zx_order = [("x", 0), ("z", 1), ("x", 1), ("x", 2), ("x", 3)]
            for kind, h in zx_order:
                for et in range(2):
                    if kind == "z":
                        nc.sync.dma_start(
                            out=zT[:, et, 1024 * h:1024 * (h + 1)],
                            in_=zt_ext[et, :, 1024 * h:1024 * (h + 1)])
                    else:
                        nc.sync.dma_start(
                            out=xT[:, et, 1024 * h:1024 * (h + 1)],
                            in_=xt_ext[et, :, 1024 * h:1024 * (h + 1)])

            def front_block(sc):
                # K^T tile, exp-bias row bqk, and natural-V tile for
                # z columns [512sc, 512(sc+1))
                psk = ps_a.tile([128, 2 * F], f32, tag="ps_a", name="psk")
                for g in range(2):
                    for et in range(2):
                        nc.tensor.matmul(psk[:, F * g:F * (g + 1)],
                                         wk_sb[:, et, 128 * g:128 * (g + 1)],
                                         zT[:, et, 512 * sc:512 * (sc + 1)],
                                         start=(et == 0), stop=(et == 1),
                                         skip_group_check=(g == 1))
                nc.vector.tensor_copy(out=kT[:, :, 512 * sc:512 * (sc + 1)],
                                      in_=psk[:])
                # bqk = bqw . Z^T, bounced through DRAM into [128, 4] layout
                psb = ps_a.tile([1, F], f32, tag="ps_a", name="psb",
                                padded_shape=[128, 2 * F])
                for et in range(2):
                    nc.tensor.matmul(psb[:], bqw[:, et:et + 1],
                                     zT[:, et, 512 * sc:512 * (sc + 1)],
                                     start=(et == 0), stop=(et == 1))
                nc.vector.tensor_copy(out=bqk_row[:, 512 * sc:512 * (sc + 1)],
                                      in_=psb[:])
                nc.gpsimd.dma_start(
                    out=bqk_dram[512 * sc:512 * (sc + 1)],
                    in_=bqk_row[0:1, 512 * sc:512 * (sc + 1)])
                nc.gpsimd.dma_start(
                    out=bqk_t[:, 4 * sc:4 * (sc + 1)],
                    in_=bqk_dram[512 * sc:512 * (sc + 1)].rearrange(
                        "(l p) -> p l", p=128))
                psv = ps_a.tile([128, 4, E], f32, tag="ps_a", name="psv",
                                padded_shape=[128, 4, E])
                for t in range(4):
                    base = 512 * sc + 128 * t
                    for et in range(2):
                        nc.tensor.matmul(psv[:, t, :],
                                         zT[:, et, base:base + 128],
                                         wv_sb[:, et, :],
                                         start=(et == 0), stop=(et == 1),
                                         skip_group_check=(t > 0))
                nc.vector.tensor_copy(out=v_sb[:, 4 * sc:4 * sc + 4, :],
                                      in_=psv[:])

            def back_block(sc):
                # K2 = Wq'^T K^T (scores then contract X directly)
                psk2 = ps_a.tile([128, 2 * F], f32, tag="ps_a", name="psk2")
                for g in range(2):
                    for ft in range(2):
                        nc.tensor.matmul(psk2[:, F * g:F * (g + 1)],
                                         wq_sb[:, ft, 128 * g:128 * (g + 1)],
                                         kT[:, ft, 512 * sc:512 * (sc + 1)],
                                         start=(ft == 0), stop=(ft == 1),
                                         skip_group_check=(g == 1))
                nc.vector.tensor_copy(out=k2T[:, :, 512 * sc:512 * (sc + 1)],
                                      in_=psk2[:])

            def attn_pair(m, interleave=None):
                c0, c1 = 2 * m, 2 * m + 1
                n0, n1 = 2 * c0 + 2, 2 * c1 + 2
                pso0 = ps_o.tile([128, 2 * F], f32, tag="ps_o", name="pso0")
                pso1 = ps_o.tile([128, 2 * F], f32, tag="ps_o", name="pso1")
                acc = accp.tile([128, 2 * F], bf16, tag="acc", name="acc")
                nc.gpsimd.memset(acc[:], 0.0)

                # the last pair runs its two c1-only tiles first so the
                # kernel ends on dense full-width work (short exposed tail)
                if m == 3:
                    order = list(range(n0, n1)) + list(range(n0))
                else:
                    order = list(range(n1))
                c0_last = [ll for ll in order if ll < n0][-1]
                c1_last = order[-1]
                for idx, ll in enumerate(order):
                    if interleave is not None and idx == 2:
                        interleave()
                    both = ll < n0
                    pss = ps_a.tile([128, 2 * F], f32, tag="ps_a", name="pss")
                    pt = pTp.tile([128, 2 * F], bf16, tag="pT", name="pt")
                    regions = []   # (chunk, col base, live col start)
                    for ci, base in ([(c0, 0)] if both else []) + [(c1, F)]:
                        cs = 256 if ll == 2 * ci + 1 else 0
                        regions.append((ci, base, cs))
                    lo_all = regions[0][1] + regions[0][2]
                    hi = 2 * F
                    for ci, base, cs in regions:
                        for g in range(2):
                            nc.tensor.matmul(
                                pss[:, base + cs:base + F],
                                k2T[:, g, 128 * ll:128 * (ll + 1)],
                                xT[:, g, 512 * ci + cs:512 * (ci + 1)],
                                start=(g == 0), stop=(g == 1),
                                skip_group_check=True)
                    nc.scalar.activation(out=pt[:, lo_all:hi],
                                         in_=pss[:, lo_all:hi], func=Exp,
                                         bias=bqk_t[:, ll:ll + 1])
                    # diag masks: one 256-wide parity-encoded pattern
                    for ci, base, cs in regions:
                        if ll >= 2 * ci:
                            nc.vector.tensor_mul(
                                pt[:, base + cs:base + cs + 256],
                                pt[:, base + cs:base + cs + 256],
                                maskt[:])
                    # exp row-sum accumulation (bf16, DVE; acc pre-zeroed)
                    nc.vector.tensor_add(acc[:, lo_all:hi], acc[:, lo_all:hi],
                                         pt[:, lo_all:hi])
                    # P^T @ V accumulation
                    for ci, base, cs in regions:
                        pso = pso0 if ci == c0 else pso1
                        first = (ll == (0 if ci == c0 else order[0]))
                        last = (ll == (c0_last if ci == c0 else c1_last))
                        for ft in range(2):
                            nc.tensor.matmul(
                                pso[:, F * ft + cs:F * (ft + 1)],
                                v_sb[:, ll, 128 * ft:128 * (ft + 1)],
                                pt[:, base + cs:base + F],
                                start=first, stop=last,
                                skip_group_check=True)
                    if ll == c0_last:
                        po0 = pop.tile([128, 2 * F], bf16, tag="po", name="po0")
                        nc.vector.tensor_copy(out=po0[:], in_=pso0[:])
                        for ft in range(2):
                            nc.sync.dma_start(out=out_ext[c0, ft],
                                              in_=po0[:, F * ft:F * (ft + 1)])
                        nc.sync.dma_start(out=acc_ext[c0], in_=acc[:, 0:F])
                    if ll == c1_last:
                        po1 = pop.tile([128, 2 * F], bf16, tag="po", name="po1")
                        if m == 3:
                            # kernel tail: scalar is idle here; also spread
                            # the final DMAs across the three queues so their
                            # issue overheads overlap
                            nc.scalar.activation(out=po1[:], in_=pso1[:],
                                                 func=Copy)
                            nc.sync.dma_start(out=out_ext[c1, 0],
                                              in_=po1[:, 0:F])
                            nc.scalar.dma_start(out=out_ext[c1, 1],
                                                in_=po1[:, F:2 * F])
                            nc.gpsimd.dma_start(out=acc_ext[c1],
                                                in_=acc[:, F:2 * F])
                        else:
                            nc.vector.tensor_copy(out=po1[:], in_=pso1[:])
                            for ft in range(2):
                                nc.sync.dma_start(
                                    out=out_ext[c1, ft],
                                    in_=po1[:, F * ft:F * (ft + 1)])
                            nc.sync.dma_start(out=acc_ext[c1],
                                              in_=acc[:, F:2 * F])

            front_block(0)
            front_block(1)
            back_block(0)
            front_block(2)
            back_block(1)
            attn_pair(0, interleave=lambda: front_block(3))
            attn_pair(1, interleave=lambda: back_block(2))
            attn_pair(2, interleave=lambda: back_block(3))
            attn_pair(3)

    nc.compile()
    return nc


def _get_nc():
    if "nc" not in _COMPILED:
        _COMPILED["nc"] = _build()
    return _COMPILED["nc"]


def kernel(X, Z, mask, Wq, bq, Wk, bk, Wv, bv):
    import ml_dtypes
    bf16 = ml_dtypes.bfloat16

    X = np.asarray(X, dtype=np.float32)
    Z = np.asarray(Z, dtype=np.float32)
    mask_np = np.asarray(mask)
    Wq = np.asarray(Wq, dtype=np.float32)
    Wk = np.asarray(Wk, dtype=np.float32)
    Wv = np.asarray(Wv, dtype=np.float32)
    bq = np.asarray(bq, dtype=np.float32)
    bv = np.asarray(bv, dtype=np.float32)

    causal = bool(np.array_equal(
        mask_np != 0, np.tril(np.ones((S, S), dtype=bool))))
    if not causal:
        return _numpy_ref(X, Z, mask_np, Wq, bq, Wk, np.asarray(bk), Wv, bv)

    from concourse.bass_utils import run_bass_kernel_spmd

    nc = _get_nc()

    rsqE = np.float32(1.0 / np.sqrt(E))
    # wq ships in NATURAL [f, e] layout (stationary of the K2 fold-in),
    # carrying the 1/sqrt(E) score scale; wk/wv ship transposed [e, f].
    wq_n = np.ascontiguousarray((Wq * rsqE).astype(bf16)).reshape(2, 128, E)
    wk_t = np.ascontiguousarray(Wk.T.astype(bf16)).reshape(2, 128, E)
    wv_t = np.ascontiguousarray(Wv.T.astype(bf16)).reshape(2, 128, E)
    bqw_vec = (bq * rsqE) @ Wk          # e-vector: bqk = bqw . Z^T
    bqw = np.ascontiguousarray(bqw_vec.reshape(2, 128).T.astype(bf16))
    # bk is dropped: per-q-row constant in the scores -> softmax invariant.

    u = np.arange(256)[None, :]
    x = np.arange(128)[:, None]
    masks_par = [np.ascontiguousarray((u >= 128 * p + x).astype(bf16))
                 for p in range(2)]

    in_maps = []
    for c in range(8):
        b, p = c // 2, c % 2
        xt = np.ascontiguousarray(X[b].T.astype(bf16)).reshape(2, 128, S)
        zb = Z[b].reshape(S // 128, 128, E)[p::2].reshape(SK, E)
        zt = np.ascontiguousarray(zb.T.astype(bf16)).reshape(2, 128, SK)
        in_maps.append({
            "xt": xt, "zt": zt,
            "wq": wq_n, "wk": wk_t, "wv": wv_t,
            "bqw": bqw, "masks": masks_par[p],
        })

    res = run_bass_kernel_spmd(nc, in_maps, core_ids=list(range(8)))

    out = np.empty((B, S, E), dtype=np.float32)
    for b in range(B):
        r0, r1 = res.results[2 * b], res.results[2 * b + 1]
        num = r0["out"].astype(np.float32) + r1["out"].astype(np.float32)
        den = (r0["acc"].astype(np.float32).sum(axis=1)
               + r1["acc"].astype(np.float32).sum(axis=1))  # [NCH, F]
        ob = num.transpose(0, 3, 1, 2).reshape(S, E)
        out[b] = ob / den.reshape(S, 1) + bv
    return out


def _numpy_ref(X, Z, mask, Wq, bq, Wk, bk, Wv, bv):
    q = np.einsum("bse,fe->bsf", X, Wq) + bq
    k = np.einsum("bse,fe->bsf", Z, Wk) + bk
    v = np.einsum("bse,fe->bsf", Z, Wv) + bv
    s = np.einsum("bqe,bke->bqk", q, k) / np.sqrt(np.float32(X.shape[-1]))
    s = np.where(mask == 0, -np.inf, s)
    s = s - s.max(axis=-1, keepdims=True)
    p = np.exp(s)
    p /= p.sum(axis=-1, keepdims=True)
    return np.einsum("bqk,bke->bqe", p, v).astype(np.float32)
